# revision 7
# baseline (speedup 1.0000x reference)
"""Trainium2 Bass kernel for nn_BinaryDecoderWithRegularization.

Strategy (tensor-parallel over out_features, fully embarrassingly parallel):
  - Each of 8 cores owns 96 of 768 out_features (768 of 6144 weight columns).
  - Host pre-packs (pure layout/cast, no arithmetic):
      * weight shard -> bit-major chunk tiles, bf16
      * latent.T (replicated), bf16
      * true_sum shard transposed, bf16
      * a small constant matrix folding the bit powers for true_sum
  - Device per core:
      * sigma = sigmoid(W) on ScalarE (accum_out gives per-partition sum(sigma))
      * reg:  sum min(sigma, 1-sigma) = sum(sigma) - 2*sum(relu(sigma-0.5));
        the relu-sum comes from one DVE tensor_scalar op per chunk
      * IW = sum_b sigma_b * p_b via 7 fused scalar_tensor_tensor (Horner) ops
      * diffT = IW.T @ latent.T - Pblk.T @ true_sum.T accumulated in PSUM
        (one accumulation group of 76 matmuls, bf16 operands)
      * recon partial = per-partition sum of diffT^2 (ScalarE Square + accum)
  - Host: combine tiny per-core partial sums into the 3 scalar losses.
"""

import numpy as np
import ml_dtypes

IN_F = 4096
OUT_F = 768
N_BITS = 8
B = 1024
SCALE = float(2**N_BITS - 1)
REG_WEIGHT = 0.001
N_CORES = 8

OPC = OUT_F // N_CORES      # 96 out features per core
COLS = OPC * N_BITS         # 768 weight columns per core
NKT = IN_F // 128           # 32 k-tiles of latent/weight contraction dim
NCH = 4                     # weight chunks per core
KT_PER_CH = NKT // NCH      # 8
CHF = KT_PER_CH * OPC       # 768 = free elems per bit strip in a chunk
CHW = N_BITS * CHF          # 6144 = chunk free width
TS_KT = COLS // 128         # 6 k-tiles for the true_sum contraction
LAT_G = 8                   # latent tile groups
LAT_PER_G = NKT // LAT_G    # 4 k-tiles per latent group

BF16 = ml_dtypes.bfloat16
POWERS = np.array([1, 2, 4, 8, 16, 32, 64, -128], dtype=np.float32)


def _build_nc():
    import concourse.tile as tile
    import concourse.mybir as mybir
    from concourse import bacc
    from contextlib import ExitStack

    dt = mybir.dt
    alu = mybir.AluOpType
    act = mybir.ActivationFunctionType

    nc = bacc.Bacc("TRN2", target_bir_lowering=False, debug=False)
    wbits = nc.declare_dram_parameter("wbits", [NCH, 128, CHW], dt.bfloat16, isOutput=False)
    latt = nc.declare_dram_parameter("latt", [LAT_G, 128, LAT_PER_G * B], dt.bfloat16, isOutput=False)
    tst = nc.declare_dram_parameter("tst", [2, 128, 3 * B], dt.bfloat16, isOutput=False)
    pmat = nc.declare_dram_parameter("pmat", [128, TS_KT * OPC], dt.bfloat16, isOutput=False)
    o_sig = nc.declare_dram_parameter("sig_sums", [128, NCH], dt.float32, isOutput=True)
    o_relu = nc.declare_dram_parameter("relu_sums", [128, NCH], dt.float32, isOutput=True)
    o_recon = nc.declare_dram_parameter("recon_sums", [OPC, 1], dt.float32, isOutput=True)

    with ExitStack() as ctx:
        tc = ctx.enter_context(tile.TileContext(nc))
        wpool = ctx.enter_context(tc.tile_pool(name="w", bufs=2))
        spool = ctx.enter_context(tc.tile_pool(name="sig", bufs=2))
        apool = ctx.enter_context(tc.tile_pool(name="acc", bufs=2))
        latpool = ctx.enter_context(tc.tile_pool(name="lat", bufs=LAT_G))
        tspool = ctx.enter_context(tc.tile_pool(name="ts", bufs=2))
        cpool = ctx.enter_context(tc.tile_pool(name="const", bufs=1))
        iwpool = ctx.enter_context(tc.tile_pool(name="iw", bufs=1))
        stpool = ctx.enter_context(tc.tile_pool(name="stats", bufs=1))
        sqpool = ctx.enter_context(tc.tile_pool(name="sq", bufs=1))
        pspool = ctx.enter_context(tc.tile_pool(name="ps", bufs=1, space="PSUM"))

        iw = iwpool.tile([128, NKT * OPC], dt.bfloat16)
        sig_st = stpool.tile([128, NCH], dt.float32, tag="sig_st")
        relu_st = stpool.tile([128, NCH], dt.float32, tag="relu_st")
        recon_st = stpool.tile([OPC, 1], dt.float32, tag="recon_st")
        ps = pspool.tile([OPC, 2 * 512], dt.float32)

        # --- DMA loads (emission order sets queue priority) ---
        wtiles = []
        for c in range(NCH):
            wt = wpool.tile([128, CHW], dt.bfloat16)
            nc.sync.dma_start(wt[:], wbits[c])
            wtiles.append(wt)

        lat_tiles = []
        for g in range(LAT_G):
            lt = latpool.tile([128, LAT_PER_G * B], dt.bfloat16)
            nc.sync.dma_start(lt[:], latt[g])
            lat_tiles.append(lt)

        ts_tiles = []
        for jj in range(2):
            tt = tspool.tile([128, 3 * B], dt.bfloat16)
            nc.sync.dma_start(tt[:], tst[jj])
            ts_tiles.append(tt)
        pm = cpool.tile([128, TS_KT * OPC], dt.bfloat16)
        nc.sync.dma_start(pm[:], pmat[:])

        # --- per-chunk compute ---
        for c in range(NCH):
            wt = wtiles[c]
            sig = spool.tile([128, CHW], dt.float32)
            # sigma = sigmoid(w); accum_out = per-partition sum(sigma)
            nc.scalar.activation(sig[:], wt[:], act.Sigmoid, accum_out=sig_st[:, c : c + 1])

            # reg: sum |sigma - 0.5|. GPSIMD (otherwise idle) computes the
            # shifted values into the dead w tile (bf16); DVE reduces abs-sum.
            nc.gpsimd.tensor_scalar(wt[:], sig[:], 0.5, None, alu.subtract)
            nc.vector.tensor_reduce(
                relu_st[:, c : c + 1], wt[:], mybir.AxisListType.X, alu.add,
                apply_absolute_value=True,
            )

            # Horner bit collapse: iw_chunk = sum_b sigma_b * powers[b]
            #   = s0 + 2*(s1 + 2*(s2 + ... + 2*(s6 + (-2)*s7)))
            # strips: sig[:, b*CHF:(b+1)*CHF], b-th bit plane
            def strip(b):
                return sig[:, b * CHF : (b + 1) * CHF]

            h = apool.tile([128, CHF], dt.float32, tag="hacc")
            nc.vector.scalar_tensor_tensor(
                h[:], strip(7), -2.0, strip(6), alu.mult, alu.add
            )
            for b in range(5, 0, -1):
                h2 = apool.tile([128, CHF], dt.float32, tag="hacc")
                nc.vector.scalar_tensor_tensor(
                    h2[:], h[:], 2.0, strip(b), alu.mult, alu.add
                )
                h = h2
            # final: iw = s0 + 2*h, output cast to bf16 into the iw slab
            nc.vector.scalar_tensor_tensor(
                iw[:, c * CHF : (c + 1) * CHF], h[:], 2.0, strip(0), alu.mult, alu.add
            )

            # matmul burst for this chunk's 8 k-tiles
            for ktl in range(KT_PER_CH):
                kt = c * KT_PER_CH + ktl
                g, s = kt // LAT_PER_G, kt % LAT_PER_G
                lhsT = iw[:, kt * OPC : (kt + 1) * OPC]
                for n in range(2):
                    rhs = lat_tiles[g][:, s * B + n * 512 : s * B + (n + 1) * 512]
                    nc.tensor.matmul(
                        ps[:, n * 512 : (n + 1) * 512], lhsT, rhs,
                        start=(kt == 0), stop=False,
                    )

        # true_sum matmuls: accumulate -Pblk.T @ tsT into the same psum group
        for j in range(TS_KT):
            jj, sj = j // 3, j % 3
            lhsT = pm[:, j * OPC : (j + 1) * OPC]
            for n in range(2):
                rhs = ts_tiles[jj][:, sj * B + n * 512 : sj * B + (n + 1) * 512]
                nc.tensor.matmul(
                    ps[:, n * 512 : (n + 1) * 512], lhsT, rhs,
                    start=False, stop=(j == TS_KT - 1),
                )

        # recon partial: per-partition sum over batch of diff^2
        sq = sqpool.tile([OPC, 2 * 512], dt.bfloat16)
        nc.scalar.activation(sq[:], ps[:], act.Square, accum_out=recon_st[:, 0:1])

        nc.sync.dma_start(o_sig[:], sig_st[:])
        nc.sync.dma_start(o_relu[:], relu_st[:])
        nc.sync.dma_start(o_recon[:], recon_st[:])

    nc.compile()
    return nc


def _pack_inputs(latent, true_sum, weight):
    """Host-side shard + layout/cast. Returns list of per-core input dicts."""
    # latent.T, bf16, grouped k-tiles: [8, 128, 4096] free=(s,batch)
    lt = np.ascontiguousarray(latent.T).astype(BF16)  # [4096, 1024]
    latt = np.ascontiguousarray(
        lt.reshape(LAT_G, LAT_PER_G, 128, B).transpose(0, 2, 1, 3).reshape(LAT_G, 128, LAT_PER_G * B)
    )

    # pmat: lhsT tiles for the -powers block-diagonal, [128, 6*96] free=(j,o)
    pm = np.zeros((TS_KT, 128, OPC), dtype=np.float32)
    for j in range(TS_KT):
        r = np.arange(128)
        col = j * 128 + r
        pm[j, r, col // N_BITS] = -POWERS[col % N_BITS]
    pmat = np.ascontiguousarray(pm.transpose(1, 0, 2).reshape(128, TS_KT * OPC)).astype(BF16)

    in_maps = []
    for c in range(N_CORES):
        wc = weight[:, COLS * c : COLS * (c + 1)]  # [4096, 768]
        wb = (
            wc.reshape(NCH, KT_PER_CH, 128, OPC, N_BITS)
            .transpose(0, 2, 4, 1, 3)
            .reshape(NCH, 128, CHW)
        ).astype(BF16)
        tsc = np.ascontiguousarray(true_sum[:, COLS * c : COLS * (c + 1)].T)  # [768, 1024]
        tst = (
            tsc.reshape(2, 3, 128, B).transpose(0, 2, 1, 3).reshape(2, 128, 3 * B)
        ).astype(BF16)
        in_maps.append(
            {
                "wbits": np.ascontiguousarray(wb),
                "latt": latt,
                "tst": np.ascontiguousarray(tst),
                "pmat": pmat,
            }
        )
    return in_maps


def _combine(results):
    """Host-side gather of tiny per-core partial sums -> the 3 scalars."""
    abs_sum = 0.0
    recon_sum = 0.0
    for r in results:
        abs_sum += float(np.sum(r["relu_sums"].astype(np.float64)))
        recon_sum += float(np.sum(r["recon_sums"].astype(np.float64)))
    n_w = IN_F * OUT_F * N_BITS
    # sum min(s, 1-s) = 0.5*n - sum |s - 0.5|
    reg = REG_WEIGHT * (0.5 * n_w - abs_sum) / n_w
    recon = recon_sum / (SCALE * SCALE * B * OUT_F)
    total = recon + reg
    return np.array([total, recon, reg], dtype=np.float32)


_NC_CACHE = None


def kernel(latent, true_sum, weight):
    from concourse.bass_utils import run_bass_kernel_spmd

    global _NC_CACHE
    if _NC_CACHE is None:
        _NC_CACHE = _build_nc()
    nc = _NC_CACHE

    in_maps = _pack_inputs(
        np.asarray(latent, dtype=np.float32),
        np.asarray(true_sum, dtype=np.float32),
        np.asarray(weight, dtype=np.float32),
    )
    res = run_bass_kernel_spmd(nc, in_maps, core_ids=list(range(N_CORES)))
    return _combine(res.results)


# revision 9
# speedup vs baseline: 4.5683x; 4.5683x over previous
"""Trainium2 Bass kernel for nn_BinaryDecoderWithRegularization.

Strategy (tensor-parallel over out_features, fully embarrassingly parallel):
  - Each of 8 cores owns 96 of 768 out_features (768 of 6144 weight columns).
  - Host pre-packs (pure layout/cast, no arithmetic):
      * weight shard -> bit-major chunk tiles, bf16
      * latent.T (replicated), bf16
      * true_sum shard transposed, bf16
      * a small constant matrix folding the bit powers for true_sum
  - Device per core, using sigma(w) - 0.5 = 0.5*tanh(w/2):
      * t = tanh(0.5*w) on ScalarE (bf16 out; small values -> tiny rounding)
      * reg: sum min(sigma,1-sigma) = 0.5*N - 0.5*sum|t|; sum|t| via one DVE
        abs-add tensor_reduce per chunk
      * bit collapse: T = sum_b t_b * p_b via 7 fused scalar_tensor_tensor
        (Horner) ops in bf16 (2x DVE mode); int_weights = 0.5*T - 0.5 via one
        tensor_scalar
      * diffT = IW.T @ latent.T - Pblk.T @ true_sum.T accumulated in PSUM
        (one accumulation group of 76 bf16 matmuls)
      * recon partial: per-partition sum of diffT^2 (ScalarE Square + accum)
  - Host: combine tiny per-core partial sums into the 3 scalar losses.
"""

import numpy as np
import ml_dtypes

IN_F = 4096
OUT_F = 768
N_BITS = 8
B = 1024
SCALE = float(2**N_BITS - 1)
REG_WEIGHT = 0.001
N_CORES = 8

OPC = OUT_F // N_CORES      # 96 out features per core
COLS = OPC * N_BITS         # 768 weight columns per core
NKT = IN_F // 128           # 32 k-tiles of latent/weight contraction dim
NCH = 4                     # weight chunks per core
KT_PER_CH = NKT // NCH      # 8
CHF = KT_PER_CH * OPC       # 768 = free elems per bit strip in a chunk
CHW = N_BITS * CHF          # 6144 = chunk free width
TS_KT = COLS // 128         # 6 k-tiles for the true_sum contraction
LAT_G = 8                   # latent tile groups
LAT_PER_G = NKT // LAT_G    # 4 k-tiles per latent group

BF16 = ml_dtypes.bfloat16
POWERS = np.array([1, 2, 4, 8, 16, 32, 64, -128], dtype=np.float32)


def _build_nc():
    import concourse.tile as tile
    import concourse.mybir as mybir
    from concourse import bacc
    from contextlib import ExitStack

    dt = mybir.dt
    alu = mybir.AluOpType
    act = mybir.ActivationFunctionType

    nc = bacc.Bacc("TRN2", target_bir_lowering=False, debug=False)
    wbits = nc.declare_dram_parameter("wbits", [NCH, 128, CHW], dt.bfloat16, isOutput=False)
    latt = nc.declare_dram_parameter("latt", [LAT_G, 128, LAT_PER_G * B], dt.bfloat16, isOutput=False)
    tst = nc.declare_dram_parameter("tst", [2, 128, 3 * B], dt.bfloat16, isOutput=False)
    pmat = nc.declare_dram_parameter("pmat", [128, TS_KT * OPC], dt.bfloat16, isOutput=False)
    o_abs = nc.declare_dram_parameter("abs_sums", [128, NCH], dt.float32, isOutput=True)
    o_recon = nc.declare_dram_parameter("recon_sums", [OPC, 1], dt.float32, isOutput=True)

    with ExitStack() as ctx:
        tc = ctx.enter_context(tile.TileContext(nc))
        wpool = ctx.enter_context(tc.tile_pool(name="w", bufs=2))
        tpool = ctx.enter_context(tc.tile_pool(name="tanh", bufs=3))
        hpool = ctx.enter_context(tc.tile_pool(name="hacc", bufs=2))
        latpool = ctx.enter_context(tc.tile_pool(name="lat", bufs=LAT_G))
        tspool = ctx.enter_context(tc.tile_pool(name="ts", bufs=2))
        cpool = ctx.enter_context(tc.tile_pool(name="const", bufs=1))
        iwpool = ctx.enter_context(tc.tile_pool(name="iw", bufs=1))
        stpool = ctx.enter_context(tc.tile_pool(name="stats", bufs=1))
        sqpool = ctx.enter_context(tc.tile_pool(name="sq", bufs=1))
        pspool = ctx.enter_context(tc.tile_pool(name="ps", bufs=1, space="PSUM"))

        iw = iwpool.tile([128, NKT * OPC], dt.bfloat16)
        abs_st = stpool.tile([128, NCH], dt.float32, tag="abs_st")
        recon_st = stpool.tile([OPC, 1], dt.float32, tag="recon_st")
        ps = pspool.tile([OPC, 2 * 512], dt.float32)

        # --- DMA loads (emission order sets priority) ---
        pm = cpool.tile([128, TS_KT * OPC], dt.bfloat16)
        nc.sync.dma_start(pm[:], pmat[:])

        wtiles = [None] * NCH
        lat_tiles = [None] * LAT_G

        def load_w(c):
            wtiles[c] = wpool.tile([128, CHW], dt.bfloat16, tag="wt", name=f"wt{c}")
            nc.sync.dma_start(wtiles[c][:], wbits[c])

        def load_lat(g):
            lat_tiles[g] = latpool.tile([128, LAT_PER_G * B], dt.bfloat16, tag="lt", name=f"lt{g}")
            nc.sync.dma_start(lat_tiles[g][:], latt[g])

        # interleave: weight chunks early (they gate the compute chain),
        # latent groups in consumption order
        load_w(0)
        load_w(1)
        load_lat(0)
        load_lat(1)
        load_w(2)
        load_lat(2)
        load_lat(3)
        load_w(3)
        for g in range(4, LAT_G):
            load_lat(g)

        ts_tiles = []
        for jj in range(2):
            tt = tspool.tile([128, 3 * B], dt.bfloat16)
            nc.sync.dma_start(tt[:], tst[jj])
            ts_tiles.append(tt)

        # --- per-chunk compute ---
        for c in range(NCH):
            wt = wtiles[c]
            t = tpool.tile([128, CHW], dt.bfloat16)
            # t = tanh(w/2) = 2*(sigma(w) - 0.5)
            nc.scalar.activation(t[:], wt[:], act.Tanh, scale=0.5)

            # reg: sum |t| per partition
            nc.vector.tensor_reduce(
                abs_st[:, c : c + 1], t[:], mybir.AxisListType.X, alu.add,
                apply_absolute_value=True,
            )

            # Horner bit collapse: T = sum_b t_b * powers[b]
            #   = t0 + 2*(t1 + 2*(t2 + ... + 2*(t6 + (-2)*t7)))
            # int_weights = 0.5*T - 0.5
            def strip(b):
                return t[:, b * CHF : (b + 1) * CHF]

            h = hpool.tile([128, CHF], dt.bfloat16, tag="hacc")
            nc.vector.scalar_tensor_tensor(
                h[:], strip(7), -2.0, strip(6), alu.mult, alu.add
            )
            for b in range(5, -1, -1):
                h2 = hpool.tile([128, CHF], dt.bfloat16, tag="hacc")
                nc.vector.scalar_tensor_tensor(
                    h2[:], h[:], 2.0, strip(b), alu.mult, alu.add
                )
                h = h2
            nc.vector.tensor_scalar(
                iw[:, c * CHF : (c + 1) * CHF], h[:], 0.5, 0.5, alu.mult, alu.subtract
            )

            # matmul burst for this chunk's 8 k-tiles
            for ktl in range(KT_PER_CH):
                kt = c * KT_PER_CH + ktl
                g, s = kt // LAT_PER_G, kt % LAT_PER_G
                lhsT = iw[:, kt * OPC : (kt + 1) * OPC]
                for n in range(2):
                    rhs = lat_tiles[g][:, s * B + n * 512 : s * B + (n + 1) * 512]
                    nc.tensor.matmul(
                        ps[:, n * 512 : (n + 1) * 512], lhsT, rhs,
                        start=(kt == 0), stop=False,
                    )

        # true_sum matmuls: accumulate -Pblk.T @ tsT into the same psum group
        for j in range(TS_KT):
            jj, sj = j // 3, j % 3
            lhsT = pm[:, j * OPC : (j + 1) * OPC]
            for n in range(2):
                rhs = ts_tiles[jj][:, sj * B + n * 512 : sj * B + (n + 1) * 512]
                nc.tensor.matmul(
                    ps[:, n * 512 : (n + 1) * 512], lhsT, rhs,
                    start=False, stop=(j == TS_KT - 1),
                )

        # recon partial: per-partition sum over batch of diff^2
        sq = sqpool.tile([OPC, 2 * 512], dt.bfloat16)
        nc.scalar.activation(sq[:], ps[:], act.Square, accum_out=recon_st[:, 0:1])

        nc.sync.dma_start(o_abs[:], abs_st[:])
        nc.sync.dma_start(o_recon[:], recon_st[:])

    nc.compile()
    return nc


def _pack_inputs(latent, true_sum, weight):
    """Host-side shard + layout/cast. Returns list of per-core input dicts."""
    # latent.T, bf16, grouped k-tiles: [8, 128, 4096] free=(s,batch)
    lt = np.ascontiguousarray(latent.T).astype(BF16)  # [4096, 1024]
    latt = np.ascontiguousarray(
        lt.reshape(LAT_G, LAT_PER_G, 128, B).transpose(0, 2, 1, 3).reshape(LAT_G, 128, LAT_PER_G * B)
    )

    # pmat: lhsT tiles for the -powers block-diagonal, [128, 6*96] free=(j,o)
    pm = np.zeros((TS_KT, 128, OPC), dtype=np.float32)
    for j in range(TS_KT):
        r = np.arange(128)
        col = j * 128 + r
        pm[j, r, col // N_BITS] = -POWERS[col % N_BITS]
    pmat = np.ascontiguousarray(pm.transpose(1, 0, 2).reshape(128, TS_KT * OPC)).astype(BF16)

    in_maps = []
    for c in range(N_CORES):
        wc = weight[:, COLS * c : COLS * (c + 1)]  # [4096, 768]
        wb = (
            wc.reshape(NCH, KT_PER_CH, 128, OPC, N_BITS)
            .transpose(0, 2, 4, 1, 3)
            .reshape(NCH, 128, CHW)
        ).astype(BF16)
        tsc = np.ascontiguousarray(true_sum[:, COLS * c : COLS * (c + 1)].T)  # [768, 1024]
        tst = (
            tsc.reshape(2, 3, 128, B).transpose(0, 2, 1, 3).reshape(2, 128, 3 * B)
        ).astype(BF16)
        in_maps.append(
            {
                "wbits": np.ascontiguousarray(wb),
                "latt": latt,
                "tst": np.ascontiguousarray(tst),
                "pmat": pmat,
            }
        )
    return in_maps


def _combine(results):
    """Host-side gather of tiny per-core partial sums -> the 3 scalars."""
    abs_sum = 0.0
    recon_sum = 0.0
    for r in results:
        abs_sum += float(np.sum(r["abs_sums"].astype(np.float64)))
        recon_sum += float(np.sum(r["recon_sums"].astype(np.float64)))
    n_w = IN_F * OUT_F * N_BITS
    # sum min(s, 1-s) = 0.5*n - sum|s-0.5| = 0.5*n - 0.5*sum|tanh(w/2)|
    reg = REG_WEIGHT * (0.5 * n_w - 0.5 * abs_sum) / n_w
    recon = recon_sum / (SCALE * SCALE * B * OUT_F)
    total = recon + reg
    return np.array([total, recon, reg], dtype=np.float32)


_NC_CACHE = None


def kernel(latent, true_sum, weight):
    from concourse.bass_utils import run_bass_kernel_spmd

    global _NC_CACHE
    if _NC_CACHE is None:
        _NC_CACHE = _build_nc()
    nc = _NC_CACHE

    in_maps = _pack_inputs(
        np.asarray(latent, dtype=np.float32),
        np.asarray(true_sum, dtype=np.float32),
        np.asarray(weight, dtype=np.float32),
    )
    res = run_bass_kernel_spmd(nc, in_maps, core_ids=list(range(N_CORES)))
    return _combine(res.results)


# revision 13
# speedup vs baseline: 5.3742x; 1.1764x over previous
"""Trainium2 Bass kernel for nn_BinaryDecoderWithRegularization.

Strategy (tensor-parallel over out_features, fully embarrassingly parallel):
  - Each of 8 cores owns 96 of 768 out_features (768 of 6144 weight columns).
  - Host pre-packs (pure layout/cast, no arithmetic):
      * weight shard -> bit-major chunk tiles, bf16
      * latent.T (replicated), bf16
      * true_sum shard transposed, bf16
      * a small constant matrix folding the bit powers for true_sum
  - Device per core, using sigma(w) - 0.5 = 0.5*tanh(w/2):
      * t = tanh(0.5*w) on ScalarE (bf16 out; small values -> tiny rounding)
      * reg: sum min(sigma,1-sigma) = 0.5*N - 0.5*sum|t|; sum|t| via one DVE
        abs-add tensor_reduce per chunk
      * bit collapse: T = sum_b t_b * p_b via 7 fused scalar_tensor_tensor
        (Horner) ops in bf16 (2x DVE mode); int_weights = 0.5*T - 0.5 via one
        tensor_scalar
      * diffT = IW.T @ latent.T - Pblk.T @ true_sum.T accumulated in PSUM
        (one accumulation group of 76 bf16 matmuls)
      * recon partial: per-partition sum of diffT^2 (ScalarE Square + accum)
  - Host: combine tiny per-core partial sums into the 3 scalar losses.
"""

import numpy as np
import ml_dtypes

IN_F = 4096
OUT_F = 768
N_BITS = 8
B = 1024
SCALE = float(2**N_BITS - 1)
REG_WEIGHT = 0.001
N_CORES = 8

OPC = OUT_F // N_CORES      # 96 out features per core
COLS = OPC * N_BITS         # 768 weight columns per core
NKT = IN_F // 128           # 32 k-tiles of latent/weight contraction dim
NCH = 4                     # weight chunks per core
KT_PER_CH = NKT // NCH      # 8
CHF = KT_PER_CH * OPC       # 768 = free elems per bit strip in a chunk
CHW = N_BITS * CHF          # 6144 = chunk free width
TS_KT = COLS // 128         # 6 k-tiles for the true_sum contraction
LAT_G = 8                   # latent tile groups
LAT_PER_G = NKT // LAT_G    # 4 k-tiles per latent group

BF16 = ml_dtypes.bfloat16
POWERS = np.array([1, 2, 4, 8, 16, 32, 64, -128], dtype=np.float32)


def _build_nc():
    import concourse.tile as tile
    import concourse.mybir as mybir
    from concourse import bacc
    from contextlib import ExitStack

    dt = mybir.dt
    alu = mybir.AluOpType
    act = mybir.ActivationFunctionType

    nc = bacc.Bacc("TRN2", target_bir_lowering=False, debug=False)
    wbits = nc.declare_dram_parameter("wbits", [NCH, 128, CHW], dt.bfloat16, isOutput=False)
    latt = nc.declare_dram_parameter("latt", [LAT_G, 128, LAT_PER_G * B], dt.bfloat16, isOutput=False)
    tst = nc.declare_dram_parameter("tst", [2, 128, 3 * B], dt.bfloat16, isOutput=False)
    pmat = nc.declare_dram_parameter("pmat", [128, TS_KT * OPC], dt.bfloat16, isOutput=False)
    o_abs = nc.declare_dram_parameter("abs_sums", [128, NCH], dt.float32, isOutput=True)
    o_recon = nc.declare_dram_parameter("recon_sums", [OPC, 1], dt.float32, isOutput=True)

    with ExitStack() as ctx:
        tc = ctx.enter_context(tile.TileContext(nc))
        wpool = ctx.enter_context(tc.tile_pool(name="w", bufs=4))
        tpool = ctx.enter_context(tc.tile_pool(name="tanh", bufs=4))
        hpool = ctx.enter_context(tc.tile_pool(name="hacc", bufs=2))
        latpool = ctx.enter_context(tc.tile_pool(name="lat", bufs=LAT_G))
        tspool = ctx.enter_context(tc.tile_pool(name="ts", bufs=2))
        cpool = ctx.enter_context(tc.tile_pool(name="const", bufs=1))
        iwpool = ctx.enter_context(tc.tile_pool(name="iw", bufs=1))
        stpool = ctx.enter_context(tc.tile_pool(name="stats", bufs=1))
        sqpool = ctx.enter_context(tc.tile_pool(name="sq", bufs=1))
        pspool = ctx.enter_context(tc.tile_pool(name="ps", bufs=1, space="PSUM"))

        iw = iwpool.tile([128, NKT * OPC], dt.bfloat16)
        abs_st = stpool.tile([128, NCH], dt.float32, tag="abs_st")
        recon_st = stpool.tile([OPC, 1], dt.float32, tag="recon_st")
        ps = pspool.tile([OPC, 2 * 512], dt.float32)

        # --- DMA loads (emission order sets priority) ---
        pm = cpool.tile([128, TS_KT * OPC], dt.bfloat16)
        nc.sync.dma_start(pm[:], pmat[:])

        wtiles = [None] * NCH
        lat_tiles = [None] * LAT_G

        def load_w(c):
            wtiles[c] = wpool.tile([128, CHW], dt.bfloat16, tag="wt", name=f"wt{c}")
            nc.sync.dma_start(wtiles[c][:], wbits[c])

        def load_lat(g):
            lat_tiles[g] = latpool.tile([128, LAT_PER_G * B], dt.bfloat16, tag="lt", name=f"lt{g}")
            nc.sync.dma_start(lat_tiles[g][:], latt[g])

        # weight chunks first (they gate the tanh->Horner chain), then latent
        # groups in consumption order, true_sum last (shortest dependent chain)
        for c in range(NCH):
            load_w(c)
        for g in range(LAT_G):
            load_lat(g)

        ts_tiles = []
        for jj in range(2):
            tt = tspool.tile([128, 3 * B], dt.bfloat16)
            nc.sync.dma_start(tt[:], tst[jj])
            ts_tiles.append(tt)

        # --- per-chunk compute ---
        for c in range(NCH):
            wt = wtiles[c]
            t = tpool.tile([128, CHW], dt.bfloat16)
            # t = tanh(w/2) = 2*(sigma(w) - 0.5)
            nc.scalar.activation(t[:], wt[:], act.Tanh, scale=0.5)

            # reg: sum |t| per partition. Chunks 0-2 on ScalarE (Abs act with
            # accumulator, output overwrites the dead weight tile); chunk 3 on
            # DVE after its Horner chain so IW3 completes ASAP.
            if c < 3:
                nc.scalar.activation(
                    wt[:], t[:], act.Abs, accum_out=abs_st[:, c : c + 1]
                )

            # Horner bit collapse: T = sum_b t_b * powers[b]
            #   = t0 + 2*(t1 + 2*(t2 + ... + 2*(t6 + (-2)*t7)))
            # int_weights = 0.5*T - 0.5
            def strip(b):
                return t[:, b * CHF : (b + 1) * CHF]

            h = hpool.tile([128, CHF], dt.bfloat16, tag="hacc")
            nc.vector.scalar_tensor_tensor(
                h[:], strip(7), -2.0, strip(6), alu.mult, alu.add
            )
            for b in range(5, -1, -1):
                h2 = hpool.tile([128, CHF], dt.bfloat16, tag="hacc")
                nc.vector.scalar_tensor_tensor(
                    h2[:], h[:], 2.0, strip(b), alu.mult, alu.add
                )
                h = h2
            nc.vector.tensor_scalar(
                iw[:, c * CHF : (c + 1) * CHF], h[:], 0.5, 0.5, alu.mult, alu.subtract
            )
            if c == 3:
                nc.vector.tensor_reduce(
                    abs_st[:, c : c + 1], t[:], mybir.AxisListType.X, alu.add,
                    apply_absolute_value=True,
                )

            # matmul burst for this chunk's 8 k-tiles
            for ktl in range(KT_PER_CH):
                kt = c * KT_PER_CH + ktl
                g, s = kt // LAT_PER_G, kt % LAT_PER_G
                lhsT = iw[:, kt * OPC : (kt + 1) * OPC]
                for n in range(2):
                    rhs = lat_tiles[g][:, s * B + n * 512 : s * B + (n + 1) * 512]
                    nc.tensor.matmul(
                        ps[:, n * 512 : (n + 1) * 512], lhsT, rhs,
                        start=(kt == 0), stop=False,
                    )

        # true_sum matmuls: accumulate -Pblk.T @ tsT into the same psum group
        for j in range(TS_KT):
            jj, sj = j // 3, j % 3
            lhsT = pm[:, j * OPC : (j + 1) * OPC]
            for n in range(2):
                rhs = ts_tiles[jj][:, sj * B + n * 512 : sj * B + (n + 1) * 512]
                nc.tensor.matmul(
                    ps[:, n * 512 : (n + 1) * 512], lhsT, rhs,
                    start=False, stop=(j == TS_KT - 1),
                )

        # recon partial: per-partition sum over batch of diff^2
        sq = sqpool.tile([OPC, 2 * 512], dt.bfloat16)
        nc.scalar.activation(sq[:], ps[:], act.Square, accum_out=recon_st[:, 0:1])

        nc.sync.dma_start(o_abs[:], abs_st[:])
        nc.sync.dma_start(o_recon[:], recon_st[:])

    nc.compile()
    return nc


def _pack_inputs(latent, true_sum, weight):
    """Host-side shard + layout/cast. Returns list of per-core input dicts."""
    # latent.T, bf16, grouped k-tiles: [8, 128, 4096] free=(s,batch)
    lt = np.ascontiguousarray(latent.T).astype(BF16)  # [4096, 1024]
    latt = np.ascontiguousarray(
        lt.reshape(LAT_G, LAT_PER_G, 128, B).transpose(0, 2, 1, 3).reshape(LAT_G, 128, LAT_PER_G * B)
    )

    # pmat: lhsT tiles for the -powers block-diagonal, [128, 6*96] free=(j,o)
    pm = np.zeros((TS_KT, 128, OPC), dtype=np.float32)
    for j in range(TS_KT):
        r = np.arange(128)
        col = j * 128 + r
        pm[j, r, col // N_BITS] = -POWERS[col % N_BITS]
    pmat = np.ascontiguousarray(pm.transpose(1, 0, 2).reshape(128, TS_KT * OPC)).astype(BF16)

    in_maps = []
    for c in range(N_CORES):
        wc = weight[:, COLS * c : COLS * (c + 1)]  # [4096, 768]
        wb = (
            wc.reshape(NCH, KT_PER_CH, 128, OPC, N_BITS)
            .transpose(0, 2, 4, 1, 3)
            .reshape(NCH, 128, CHW)
        ).astype(BF16)
        tsc = np.ascontiguousarray(true_sum[:, COLS * c : COLS * (c + 1)].T)  # [768, 1024]
        tst = (
            tsc.reshape(2, 3, 128, B).transpose(0, 2, 1, 3).reshape(2, 128, 3 * B)
        ).astype(BF16)
        in_maps.append(
            {
                "wbits": np.ascontiguousarray(wb),
                "latt": latt,
                "tst": np.ascontiguousarray(tst),
                "pmat": pmat,
            }
        )
    return in_maps


def _combine(results):
    """Host-side gather of tiny per-core partial sums -> the 3 scalars."""
    abs_sum = 0.0
    recon_sum = 0.0
    for r in results:
        abs_sum += float(np.sum(r["abs_sums"].astype(np.float64)))
        recon_sum += float(np.sum(r["recon_sums"].astype(np.float64)))
    n_w = IN_F * OUT_F * N_BITS
    # sum min(s, 1-s) = 0.5*n - sum|s-0.5| = 0.5*n - 0.5*sum|tanh(w/2)|
    reg = REG_WEIGHT * (0.5 * n_w - 0.5 * abs_sum) / n_w
    recon = recon_sum / (SCALE * SCALE * B * OUT_F)
    total = recon + reg
    return np.array([total, recon, reg], dtype=np.float32)


_NC_CACHE = None


def kernel(latent, true_sum, weight):
    from concourse.bass_utils import run_bass_kernel_spmd

    global _NC_CACHE
    if _NC_CACHE is None:
        _NC_CACHE = _build_nc()
    nc = _NC_CACHE

    in_maps = _pack_inputs(
        np.asarray(latent, dtype=np.float32),
        np.asarray(true_sum, dtype=np.float32),
        np.asarray(weight, dtype=np.float32),
    )
    res = run_bass_kernel_spmd(nc, in_maps, core_ids=list(range(N_CORES)))
    return _combine(res.results)


# revision 16
# speedup vs baseline: 5.8514x; 1.0888x over previous
"""Trainium2 Bass kernel for nn_BinaryDecoderWithRegularization.

Strategy (tensor-parallel over out_features, fully embarrassingly parallel):
  - Each of 8 cores owns 96 of 768 out_features (768 of 6144 weight columns).
  - Host pre-packs (pure layout/cast, no arithmetic):
      * weight shard -> bit-major chunk tiles, bf16
      * latent.T (replicated), bf16
      * true_sum shard transposed, bf16
      * a small constant matrix folding the bit powers for true_sum
  - Device per core, using sigma(w) - 0.5 = 0.5*tanh(w/2):
      * t = tanh(0.5*w) on ScalarE (bf16 out; small values -> tiny rounding)
      * reg: sum min(sigma,1-sigma) = 0.5*N - 0.5*sum|t|; sum|t| via one DVE
        abs-add tensor_reduce per chunk
      * bit collapse: T = sum_b t_b * p_b via 7 fused scalar_tensor_tensor
        (Horner) ops in bf16 (2x DVE mode); int_weights = 0.5*T - 0.5 via one
        tensor_scalar
      * diffT = IW.T @ latent.T - Pblk.T @ true_sum.T accumulated in PSUM
        (one accumulation group of 76 bf16 matmuls)
      * recon partial: per-partition sum of diffT^2 (ScalarE Square + accum)
  - Host: combine tiny per-core partial sums into the 3 scalar losses.
"""

import numpy as np
import ml_dtypes

IN_F = 4096
OUT_F = 768
N_BITS = 8
B = 1024
SCALE = float(2**N_BITS - 1)
REG_WEIGHT = 0.001
N_CORES = 8

OPC = OUT_F // N_CORES      # 96 out features per core
COLS = OPC * N_BITS         # 768 weight columns per core
NKT = IN_F // 128           # 32 k-tiles of latent/weight contraction dim
NCH = 4                     # weight chunks per core
KT_PER_CH = NKT // NCH      # 8
CHF = KT_PER_CH * OPC       # 768 = free elems per bit strip in a chunk
CHW = N_BITS * CHF          # 6144 = chunk free width
TS_KT = COLS // 128         # 6 k-tiles for the true_sum contraction
LAT_G = 8                   # latent tile groups
LAT_PER_G = NKT // LAT_G    # 4 k-tiles per latent group

BF16 = ml_dtypes.bfloat16
POWERS = np.array([1, 2, 4, 8, 16, 32, 64, -128], dtype=np.float32)


def _build_nc():
    import concourse.tile as tile
    import concourse.mybir as mybir
    from concourse import bacc
    from contextlib import ExitStack

    dt = mybir.dt
    alu = mybir.AluOpType
    act = mybir.ActivationFunctionType

    nc = bacc.Bacc("TRN2", target_bir_lowering=False, debug=False)
    wbits = nc.declare_dram_parameter("wbits", [NCH, 128, CHW], dt.bfloat16, isOutput=False)
    latt = nc.declare_dram_parameter("latt", [LAT_G, 128, LAT_PER_G * B], dt.bfloat16, isOutput=False)
    tst = nc.declare_dram_parameter("tst", [2, 128, 3 * B], dt.bfloat16, isOutput=False)
    pmat = nc.declare_dram_parameter("pmat", [128, TS_KT * OPC], dt.bfloat16, isOutput=False)
    o_abs = nc.declare_dram_parameter("abs_sums", [128, NCH], dt.float32, isOutput=True)
    o_recon = nc.declare_dram_parameter("recon_sums", [OPC, 1], dt.float32, isOutput=True)

    with ExitStack() as ctx:
        tc = ctx.enter_context(tile.TileContext(nc))
        wpool = ctx.enter_context(tc.tile_pool(name="w", bufs=4))
        tpool = ctx.enter_context(tc.tile_pool(name="tanh", bufs=4))
        hpool = ctx.enter_context(tc.tile_pool(name="hacc", bufs=2))
        latpool = ctx.enter_context(tc.tile_pool(name="lat", bufs=LAT_G))
        tspool = ctx.enter_context(tc.tile_pool(name="ts", bufs=2))
        cpool = ctx.enter_context(tc.tile_pool(name="const", bufs=1))
        iwpool = ctx.enter_context(tc.tile_pool(name="iw", bufs=1))
        stpool = ctx.enter_context(tc.tile_pool(name="stats", bufs=1))
        sqpool = ctx.enter_context(tc.tile_pool(name="sq", bufs=1))
        pspool = ctx.enter_context(tc.tile_pool(name="ps", bufs=1, space="PSUM"))

        iw = iwpool.tile([128, NKT * OPC], dt.bfloat16)
        abs_st = stpool.tile([128, NCH], dt.float32, tag="abs_st")
        recon_st = stpool.tile([OPC, 1], dt.float32, tag="recon_st")
        ps = pspool.tile([OPC, 2 * 512], dt.float32)

        # --- DMA loads (emission order sets priority) ---
        pm = cpool.tile([128, TS_KT * OPC], dt.bfloat16)
        nc.sync.dma_start(pm[:], pmat[:])

        wtiles = [None] * NCH
        lat_tiles = [None] * LAT_G

        def load_w(c):
            wtiles[c] = wpool.tile([128, CHW], dt.bfloat16, tag="wt", name=f"wt{c}")
            nc.sync.dma_start(wtiles[c][:], wbits[c])

        def load_lat(g):
            lat_tiles[g] = latpool.tile([128, LAT_PER_G * B], dt.bfloat16, tag="lt", name=f"lt{g}")
            nc.sync.dma_start(lat_tiles[g][:], latt[g])

        # weight chunks first (they gate the tanh->Horner chain), then latent
        # groups in consumption order, true_sum last (shortest dependent chain)
        for c in range(NCH):
            load_w(c)
        for g in range(LAT_G):
            load_lat(g)

        ts_tiles = []
        for jj in range(2):
            tt = tspool.tile([128, 3 * B], dt.bfloat16)
            nc.sync.dma_start(tt[:], tst[jj])
            ts_tiles.append(tt)

        # --- phase A: all tanh ops first (they gate the whole chain; emission
        # order defines scheduler priority, so nothing may jump ahead) ---
        t_tiles = []
        for c in range(NCH):
            t = tpool.tile([128, CHW], dt.bfloat16, tag="t", name=f"t{c}")
            # t = tanh(w/2) = 2*(sigma(w) - 0.5)
            nc.scalar.activation(t[:], wtiles[c][:], act.Tanh, scale=0.5)
            t_tiles.append(t)

        # --- phase B: per-chunk Horner collapse + matmul burst ---
        for c in range(NCH):
            t = t_tiles[c]

            # Horner bit collapse: T = sum_b t_b * powers[b]
            #   = t0 + 2*(t1 + 2*(t2 + ... + 2*(t6 + (-2)*t7)))
            # int_weights = 0.5*T - 0.5
            def strip(b):
                return t[:, b * CHF : (b + 1) * CHF]

            h = hpool.tile([128, CHF], dt.bfloat16, tag="hacc")
            nc.vector.scalar_tensor_tensor(
                h[:], strip(7), -2.0, strip(6), alu.mult, alu.add
            )
            for b in range(5, -1, -1):
                h2 = hpool.tile([128, CHF], dt.bfloat16, tag="hacc")
                nc.vector.scalar_tensor_tensor(
                    h2[:], h[:], 2.0, strip(b), alu.mult, alu.add
                )
                h = h2
            nc.vector.tensor_scalar(
                iw[:, c * CHF : (c + 1) * CHF], h[:], 0.5, 0.5, alu.mult, alu.subtract
            )

            # matmul burst for this chunk's 8 k-tiles
            for ktl in range(KT_PER_CH):
                kt = c * KT_PER_CH + ktl
                g, s = kt // LAT_PER_G, kt % LAT_PER_G
                lhsT = iw[:, kt * OPC : (kt + 1) * OPC]
                for n in range(2):
                    rhs = lat_tiles[g][:, s * B + n * 512 : s * B + (n + 1) * 512]
                    nc.tensor.matmul(
                        ps[:, n * 512 : (n + 1) * 512], lhsT, rhs,
                        start=(kt == 0), stop=False,
                    )

        # --- phase C: reg abs-sums (off the critical path; chunks 0-2 on
        # ScalarE with the accumulator, chunk 3 on DVE which is idle by then;
        # Abs output overwrites the dead weight tiles) ---
        for c in range(3):
            nc.scalar.activation(
                wtiles[c][:], t_tiles[c][:], act.Abs, accum_out=abs_st[:, c : c + 1]
            )
        nc.vector.tensor_reduce(
            abs_st[:, 3:4], t_tiles[3][:], mybir.AxisListType.X, alu.add,
            apply_absolute_value=True,
        )

        # true_sum matmuls: accumulate -Pblk.T @ tsT into the same psum group
        for j in range(TS_KT):
            jj, sj = j // 3, j % 3
            lhsT = pm[:, j * OPC : (j + 1) * OPC]
            for n in range(2):
                rhs = ts_tiles[jj][:, sj * B + n * 512 : sj * B + (n + 1) * 512]
                nc.tensor.matmul(
                    ps[:, n * 512 : (n + 1) * 512], lhsT, rhs,
                    start=False, stop=(j == TS_KT - 1),
                )

        # recon partial: per-partition sum over batch of diff^2
        sq = sqpool.tile([OPC, 2 * 512], dt.bfloat16)
        nc.scalar.activation(sq[:], ps[:], act.Square, accum_out=recon_st[:, 0:1])

        nc.sync.dma_start(o_abs[:], abs_st[:])
        nc.sync.dma_start(o_recon[:], recon_st[:])

    nc.compile()
    return nc


def _pack_inputs(latent, true_sum, weight):
    """Host-side shard + layout/cast. Returns list of per-core input dicts."""
    # latent.T, bf16, grouped k-tiles: [8, 128, 4096] free=(s,batch)
    lt = np.ascontiguousarray(latent.T).astype(BF16)  # [4096, 1024]
    latt = np.ascontiguousarray(
        lt.reshape(LAT_G, LAT_PER_G, 128, B).transpose(0, 2, 1, 3).reshape(LAT_G, 128, LAT_PER_G * B)
    )

    # pmat: lhsT tiles for the -powers block-diagonal, [128, 6*96] free=(j,o)
    pm = np.zeros((TS_KT, 128, OPC), dtype=np.float32)
    for j in range(TS_KT):
        r = np.arange(128)
        col = j * 128 + r
        pm[j, r, col // N_BITS] = -POWERS[col % N_BITS]
    pmat = np.ascontiguousarray(pm.transpose(1, 0, 2).reshape(128, TS_KT * OPC)).astype(BF16)

    in_maps = []
    for c in range(N_CORES):
        wc = weight[:, COLS * c : COLS * (c + 1)]  # [4096, 768]
        wb = (
            wc.reshape(NCH, KT_PER_CH, 128, OPC, N_BITS)
            .transpose(0, 2, 4, 1, 3)
            .reshape(NCH, 128, CHW)
        ).astype(BF16)
        tsc = np.ascontiguousarray(true_sum[:, COLS * c : COLS * (c + 1)].T)  # [768, 1024]
        tst = (
            tsc.reshape(2, 3, 128, B).transpose(0, 2, 1, 3).reshape(2, 128, 3 * B)
        ).astype(BF16)
        in_maps.append(
            {
                "wbits": np.ascontiguousarray(wb),
                "latt": latt,
                "tst": np.ascontiguousarray(tst),
                "pmat": pmat,
            }
        )
    return in_maps


def _combine(results):
    """Host-side gather of tiny per-core partial sums -> the 3 scalars."""
    abs_sum = 0.0
    recon_sum = 0.0
    for r in results:
        abs_sum += float(np.sum(r["abs_sums"].astype(np.float64)))
        recon_sum += float(np.sum(r["recon_sums"].astype(np.float64)))
    n_w = IN_F * OUT_F * N_BITS
    # sum min(s, 1-s) = 0.5*n - sum|s-0.5| = 0.5*n - 0.5*sum|tanh(w/2)|
    reg = REG_WEIGHT * (0.5 * n_w - 0.5 * abs_sum) / n_w
    recon = recon_sum / (SCALE * SCALE * B * OUT_F)
    total = recon + reg
    return np.array([total, recon, reg], dtype=np.float32)


_NC_CACHE = None


def kernel(latent, true_sum, weight):
    from concourse.bass_utils import run_bass_kernel_spmd

    global _NC_CACHE
    if _NC_CACHE is None:
        _NC_CACHE = _build_nc()
    nc = _NC_CACHE

    in_maps = _pack_inputs(
        np.asarray(latent, dtype=np.float32),
        np.asarray(true_sum, dtype=np.float32),
        np.asarray(weight, dtype=np.float32),
    )
    res = run_bass_kernel_spmd(nc, in_maps, core_ids=list(range(N_CORES)))
    return _combine(res.results)
